# revision 2
# baseline (speedup 1.0000x reference)
"""MixIT loss kernel for Trainium2 (8 NeuronCores, Bass/Tile) — v2.

Math: reference computes, for each of 16 assignment combinations k,
    mix[k,b,c,t] = sum_s A[k,c,s] * x[b,s,t]        (A tiny [16,2,4])
    loss[k] = sum_b [ snr(mix[k,b,0], m1[b]) + snr(mix[k,b,1], m2[b]) ]
and returns (argmin_k, min_k).  Everything reduces to the 6x6 Gram matrix
of the per-batch streams {x_0..x_3, m1, m2} over T=64000; the device
computes pairwise dot products, the host finishes the 16-way argmin.

v2 layout per core (4 batches = 24 streams; T = 128 partitions x 500 cols):
 - m1/m2 land as full-row DMAs (2000B descriptors, ~94% HBM eff) on the
   scalar HWDGE ring; x lands in T-chunks on the sync ring so the PE can
   chase.
 - DVE + ACT re-layout za[128, j, c] (f32) -> zb[128, c, j] (bf16, lane
   stride 32) with contiguous reads / strided writes, casting on the fly.
 - PE: bf16 operands zb[:, 4g:4g+4, :] = [128, 128] (24 real + 8 junk
   lanes) -> FWL fast weight load; 125 accumulating matmuls into 2 PSUM
   banks; junk lanes produce junk Gram entries the host ignores.
 - Host sums diagonal 32x32 blocks: G[j,k] = sum_f out[32f+j, 32f+k].
"""

import itertools
import sys

import numpy as np

if "/opt/trn_rl_repo" not in sys.path:
    sys.path.insert(0, "/opt/trn_rl_repo")

N_CORES = 8
B = 32               # full batch
S = 4                # estimated sources
T = 64000
BL = B // N_CORES    # batches per core = 4
NJ = 6 * BL          # real streams per core = 24 (16 x, 4 m1, 4 m2)
NJP = 32             # padded lane count (FWL wants 128-wide stationary)
P = 128
COLS = T // P        # 500
FG = 4               # T-cols fused per matmul: FG*NJP = 128
# x T-chunks (cols, each % FG == 0, sum == COLS).  Tapered: big chunks
# amortize DMA descriptors, the small last chunk shrinks the PE tail.
X_CHUNKS = (168, 140, 108, 64, 20)
assert sum(X_CHUNKS) == COLS and all(c % FG == 0 for c in X_CHUNKS)
SNR_MAX = 30.0

_CACHE = {}
LAST_RESULTS = None  # BassKernelResults of the most recent run (for test harness)


def _build_nc():
    from concourse import bacc, bass, tile
    import concourse.mybir as mybir

    nc = bacc.Bacc("TRN2", target_bir_lowering=False, debug=False,
                   num_devices=N_CORES)
    f32 = mybir.dt.float32
    bf16 = mybir.dt.bfloat16
    x = nc.dram_tensor("x", [BL, S, T], f32, kind="ExternalInput")
    m1 = nc.dram_tensor("m1", [BL, T], f32, kind="ExternalInput")
    m2 = nc.dram_tensor("m2", [BL, T], f32, kind="ExternalInput")
    g = nc.dram_tensor("g", [2, P, P], f32, kind="ExternalOutput")

    n_groups = COLS // FG                      # 125 matmuls
    grp_a_end = sum(X_CHUNKS[:-1]) // FG       # bank A: all but last chunk

    with tile.TileContext(nc) as tc:
        with (
            tc.tile_pool(name="za", bufs=1) as zapool,
            tc.tile_pool(name="zb", bufs=1) as zbpool,
            tc.tile_pool(name="ps", bufs=1, space=bass.MemorySpace.PSUM) as psp,
            tc.tile_pool(name="o", bufs=1) as opool,
        ):
            za = zapool.tile([P, NJ, COLS], f32, tag="za")
            zb = zbpool.tile([P, COLS, NJP], bf16, tag="zb")
            acc_a = psp.tile([P, P], f32, tag="pa")
            acc_b = psp.tile([P, P], f32, tag="pb")

            # Junk lanes 24:32 feed the matmul; zero them once (gpsimd is
            # otherwise idle) so no NaNs/denormals hit the PE.
            nc.gpsimd.memset(zb[:, :, NJ:NJP], 0.0)

            xr = x.ap().rearrange("b s (p c) -> p (b s) c", p=P)
            # m1/m2: full 500-col rows = 2000B descriptors, ~94% HBM eff.
            nc.scalar.dma_start(
                out=za[:, 16:20, :],
                in_=m1.ap().rearrange("b (p c) -> p b c", p=P))
            nc.scalar.dma_start(
                out=za[:, 20:24, :],
                in_=m2.ap().rearrange("b (p c) -> p b c", p=P))
            # x: T-chunked on the sync ring, 2 DMAs (8 streams) per chunk.
            c0 = 0
            for cq in X_CHUNKS:
                nc.sync.dma_start(out=za[:, 0:8, c0:c0 + cq],
                                  in_=xr[:, 0:8, c0:c0 + cq])
                nc.sync.dma_start(out=za[:, 8:16, c0:c0 + cq],
                                  in_=xr[:, 8:16, c0:c0 + cq])
                c0 += cq

            # Re-layout + cast: iterate j-outer / c-inner so reads stream
            # contiguous f32 runs; writes scatter bf16 at stride 64B.
            def cp(eng, j0, j1, c0, c1):
                dst = zb[:, c0:c1, j0:j1].transpose([0, 2, 1])
                src = za[:, j0:j1, c0:c1]
                if eng is nc.scalar:
                    eng.copy(dst, src)
                else:
                    eng.tensor_copy(dst, src)

            cp(nc.vector, 16, 20, 0, COLS)   # m1 lanes, land earliest
            cp(nc.scalar, 20, 24, 0, COLS)   # m2 lanes
            c0 = 0
            for cq in X_CHUNKS:
                cp(nc.vector, 0, 8, c0, c0 + cq)
                cp(nc.scalar, 8, 16, c0, c0 + cq)
                c0 += cq

            for grp in range(n_groups):
                op = zb[:, FG * grp:FG * (grp + 1), :]
                acc = acc_a if grp < grp_a_end else acc_b
                nc.tensor.matmul(
                    acc[:, :], op, op,
                    start=(grp == 0 or grp == grp_a_end),
                    stop=(grp == grp_a_end - 1 or grp == n_groups - 1),
                )
                if grp == grp_a_end - 1:
                    # bank A done: drain it while the PE runs the tail chunk
                    gout_a = opool.tile([P, P], f32, tag="oa")
                    nc.vector.tensor_copy(gout_a[:, :], acc_a[:, :])
                    nc.sync.dma_start(out=g.ap()[0], in_=gout_a[:, :])
            gout_b = opool.tile([P, P], f32, tag="ob")
            nc.vector.tensor_copy(gout_b[:, :], acc_b[:, :])
            nc.sync.dma_start(out=g.ap()[1], in_=gout_b[:, :])
    nc.compile()
    return nc


def _get_nc():
    if "nc" not in _CACHE:
        _CACHE["nc"] = _build_nc()
    return _CACHE["nc"]


def _finish_host(grams: np.ndarray):
    """grams: [N_CORES, 2, 128, 128] per-core PE blocks -> (argmin, min)."""
    # Collapse banks and the fused T-chunk axis:
    # G[j,k] = sum_banks sum_f out[32f+j, 32f+k], j,k in [0,24).
    g6 = grams.reshape(N_CORES, 2, FG, NJP, FG, NJP).astype(np.float64)
    g24 = np.einsum("cnfjfk->cjk", g6)[:, :NJ, :NJ]

    # Per full-batch index b: core c = b // BL, local l = b % BL.
    # Stream layout per core: x_(l,s) at 4*l+s, m1_l at 16+l, m2_l at 20+l.
    Gxx = np.empty((B, S, S), np.float64)   # sum_t x_s x_s'
    C1 = np.empty((B, S), np.float64)       # sum_t x_s m1
    C2 = np.empty((B, S), np.float64)
    M1 = np.empty((B,), np.float64)         # sum_t m1^2
    M2 = np.empty((B,), np.float64)
    for b in range(B):
        c, l = divmod(b, BL)
        gm = g24[c]
        xs = slice(S * l, S * l + S)
        Gxx[b] = gm[xs, xs]
        C1[b] = gm[xs, 16 + l]
        C2[b] = gm[xs, 20 + l]
        M1[b] = gm[16 + l, 16 + l]
        M2[b] = gm[20 + l, 20 + l]

    combos = np.array(list(itertools.product([0, 1], repeat=S)), np.float64)
    losses = np.zeros(len(combos), np.float64)
    with np.errstate(divide="ignore"):
        for w, cc, mm in ((combos, C1, M1), (1.0 - combos, C2, M2)):
            bq = np.einsum("ks,bst,kt->kb", w, Gxx, w)        # sum_t y^2
            aq = bq - 2.0 * (w @ cc.T) + mm[None, :]          # sum_t (y-m)^2
            losses += np.sum(10.0 * np.log10(aq + SNR_MAX * bq)
                             - 10.0 * np.log10(bq), axis=1)
    k = int(np.argmin(losses))
    return np.int32(k), np.float32(losses[k])


def _ensure_trace_hook_safe():
    """If BASS_TRACE is set but this image lacks antenv.axon_hooks, install a
    null hook module so run_bass_kernel_spmd degrades to an untraced run
    instead of crashing on the import."""
    try:
        import antenv.axon_hooks  # noqa: F401
    except ImportError:
        import types

        stub = types.ModuleType("antenv.axon_hooks")
        stub.get_axon_ntff_profile_hook = lambda: None
        stub.set_axon_ntff_profile_hook = lambda h: None
        sys.modules["antenv.axon_hooks"] = stub


def kernel(estimated_sources: np.ndarray, m1: np.ndarray, m2: np.ndarray):
    global LAST_RESULTS
    _ensure_trace_hook_safe()
    from concourse.bass_utils import run_bass_kernel_spmd

    x = np.ascontiguousarray(estimated_sources, dtype=np.float32)
    m1 = np.ascontiguousarray(m1, dtype=np.float32)
    m2 = np.ascontiguousarray(m2, dtype=np.float32)

    in_maps = []
    for c in range(N_CORES):
        sl = slice(BL * c, BL * (c + 1))
        in_maps.append({
            "x": np.ascontiguousarray(x[sl]),
            "m1": np.ascontiguousarray(m1[sl]),
            "m2": np.ascontiguousarray(m2[sl]),
        })

    nc = _get_nc()
    LAST_RESULTS = run_bass_kernel_spmd(nc, in_maps, list(range(N_CORES)))
    grams = np.stack([LAST_RESULTS.results[c]["g"] for c in range(N_CORES)])
    return _finish_host(grams)
